# revision 25
# baseline (speedup 1.0000x reference)
"""Distributed causal-attention kernel for one TRN2 chip (8 NeuronCores).

Sharding (hardcoded): core i handles batch i//4 and head-group i%4
(2 heads of 8, head_dim 128).  Each core:
  RMSNorm(x_b) -> QKV proj (its heads) -> causal attention -> O^T
  -> per-head 8-core AllToAll (head-shards -> seq-shards, both batches)
  -> output projection for a 256-row slice of each batch.
Host passes weights pre-transposed ([in, out] layout) with gamma and the
attention scale folded in, and gathers the 8 disjoint slices.
"""

import numpy as np

import concourse.bass as bass
import concourse.mybir as mybir
import concourse.tile as tile
from concourse import bacc
from concourse.bass_utils import run_bass_kernel_spmd
from concourse.masks import make_identity, make_causal_mask

F32 = mybir.dt.float32
BF = mybir.dt.bfloat16
AX = mybir.AxisListType.X
AF = mybir.ActivationFunctionType

S = 2048          # sequence length
D = 1024          # model dim
DH = 128          # head dim
HC = 2            # heads per core
FQKV = 3 * HC * DH  # 768 qkv cols per core (pre-transposed layout)
P = 128
SB = S // P       # 16 seq blocks
KD = D // P       # 8 d blocks
SA = float(DH) ** -0.5
NEG = -30000.0    # causal mask bias (exp underflows to exactly 0)


def _body(tc):
    nc = tc.nc
    x_ext = nc.declare_dram_parameter("x", [S, D], BF, isOutput=False)
    wqkv_ext = nc.declare_dram_parameter("w_qkvT", [D, FQKV], BF, isOutput=False)
    wout_ext = nc.declare_dram_parameter("w_outT", [D, D], BF, isOutput=False)
    out_ext = nc.declare_dram_parameter("out", [S // 4, D], F32, isOutput=True)

    from contextlib import ExitStack
    with ExitStack() as ctx:
        wpool = ctx.enter_context(tc.tile_pool(name="wpool", bufs=1))
        if True:
            wqkvT = wpool.tile([P, KD, FQKV], BF)
            nc.scalar.dma_start(
                wqkvT, wqkv_ext.ap().rearrange("(o p) f -> p o f", p=P))
            woT = wpool.tile([P, KD, D], BF)
            nc.scalar.dma_start(
                woT, wout_ext.ap().rearrange("(o p) f -> p o f", p=P))
            const = ctx.enter_context(tc.tile_pool(name="const", bufs=1))
            dram = ctx.enter_context(tc.tile_pool(name="dram", bufs=1, space="DRAM"))
            big = ctx.enter_context(tc.tile_pool(name="big", bufs=1))
            xload = ctx.enter_context(tc.tile_pool(name="xload", bufs=6))
            cast = ctx.enter_context(tc.tile_pool(name="cast", bufs=3))
            stat = ctx.enter_context(tc.tile_pool(name="stat", bufs=8))
            ppool = ctx.enter_context(tc.tile_pool(name="ppool", bufs=6))
            ptp = ctx.enter_context(tc.tile_pool(name="ptp", bufs=3))
            ps_mm = ctx.enter_context(tc.tile_pool(name="ps_mm", bufs=2, space="PSUM"))
            ps_s = ctx.enter_context(tc.tile_pool(name="ps_s", bufs=2, space="PSUM"))
            ps_o = ctx.enter_context(tc.tile_pool(name="ps_o", bufs=2, space="PSUM"))
            ps_l = ctx.enter_context(tc.tile_pool(name="ps_l", bufs=2, space="PSUM"))

            # ---- constants ----
            ident = const.tile([P, P], BF)
            make_identity(nc, ident)
            masksT = []
            for t in range(4):
                mk = const.tile([P, 512], BF, tag=f"maskT{t}")
                if t > 0:
                    nc.gpsimd.memset(mk[:, : t * P], NEG)
                # keep (0) where i >= j inside the diagonal block
                nc.gpsimd.memset(mk[:, t * P:(t + 1) * P], 0.0)
                nc.gpsimd.affine_select(
                    out=mk[:, t * P:(t + 1) * P],
                    in_=mk[:, t * P:(t + 1) * P],
                    compare_op=mybir.AluOpType.is_ge,
                    fill=NEG, base=0,
                    pattern=[[1, P]], channel_multiplier=-1)
                if t < 3:
                    nc.gpsimd.memset(mk[:, (t + 1) * P:], 0.0)
                masksT.append(mk)

            # ---- per 512-chunk: norm -> transpose -> QKV -> V ----
            xn_dram = dram.tile([S, D], BF)
            xnT = big.tile([P, KD, S], BF)
            qkvT = big.tile([P, 6, S], BF)
            v_sb = big.tile([P, SB, HC * DH], BF)
            for c in range(4):
                # norm: xn = x * (32/||x||) for s-blocks of this chunk
                xts = []
                ssg = stat.tile([P, 4], F32, tag="ssg")
                for j in range(4):
                    si = c * 4 + j
                    xt = xload.tile([P, D], BF, tag="xt")
                    nc.sync.dma_start(xt, x_ext[si * P:(si + 1) * P, :])
                    sq = cast.tile([P, D], BF, tag="sq")
                    nc.scalar.activation(sq, xt, AF.Square,
                                         accum_out=ssg[:, j:j + 1])
                    xts.append(xt)
                slg = stat.tile([P, 4], F32, tag="slg")
                nc.scalar.activation(slg, ssg, AF.Sqrt, scale=1.0 / D)
                scg = stat.tile([P, 4], F32, tag="scg")
                nc.vector.reciprocal(scg, slg)
                for j in range(4):
                    si = c * 4 + j
                    xnb = cast.tile([P, D], BF, tag="xnb")
                    nc.vector.tensor_scalar_mul(xnb, xts[j], scg[:, j:j + 1])
                    nc.scalar.dma_start(xn_dram[si * P:(si + 1) * P, :], xnb)
                # transpose chunk back: xnT [d_inner, d_outer, s]
                for k in range(KD):
                    nc.sync.dma_start_transpose(
                        xnT[:, k, c * 512:(c + 1) * 512],
                        xn_dram[c * 512:(c + 1) * 512, k * P:(k + 1) * P])
                # QKV projection for this chunk
                for fb in range(6):
                    pm = ps_mm.tile([P, 512], F32, tag="pm")
                    for k in range(KD):
                        nc.tensor.matmul(
                            pm, wqkvT[:, k, fb * P:(fb + 1) * P],
                            xnT[:, k, c * 512:(c + 1) * 512],
                            start=(k == 0), stop=(k == KD - 1))
                    nc.vector.tensor_copy(qkvT[:, fb, c * 512:(c + 1) * 512], pm)
                # V natural layout for this chunk
                for h in range(HC):
                    pst = ps_mm.tile([P, 512], BF, tag="pm")
                    for j in range(4):
                        sb = c * 4 + j
                        nc.tensor.transpose(
                            pst[:, j * P:(j + 1) * P],
                            qkvT[:, 4 + h, sb * P:(sb + 1) * P], ident)
                    nc.vector.tensor_copy(
                        v_sb[:, c * 4:(c + 1) * 4, h * DH:(h + 1) * DH],
                        pst.rearrange("p (j q) -> p j q", j=4))

            # ---- causal attention (S^T layout); per-head AllToAll ----
            # S^T chunk = matmul(lhsT=K-block [dh,128j], rhs=Q-chunk [dh,512i])
            # -> psum [128 j, 512 i]; exp -> PT_sb bf16 (PV-ready, no PE
            # transposes); l via ones-matmul; causal mask added in PSUM via
            # identity-matmul; 1/l applied to O^T via ones-broadcast.
            ones_bf = const.tile([P, 1], BF)
            nc.vector.memset(ones_bf, 1.0)
            ones_row = const.tile([1, P], F32)
            nc.vector.memset(ones_row, 1.0)
            import os
            A2DT = F32 if os.environ.get("KA2AF32") else BF
            a2a_ins = [dram.tile([8 * DH, 256], A2DT, tag=f"a2ai{h}", name=f"a2ai{h}")
                       for h in range(HC)]
            a2a_outs = [dram.tile([8 * DH, 256], A2DT, tag=f"a2ao{h}", name=f"a2ao{h}")
                        for h in range(HC)]
            for h in range(HC):
                for a in range(4):            # 512-row super-blocks of i
                    po = ps_o.tile([P, 512], F32)
                    lp = ps_l.tile([1, 512], F32)
                    nj = 4 * (a + 1)
                    for jb in range(nj):
                        t = jb - 4 * a
                        ps = ps_s.tile([P, 512], F32)
                        nc.tensor.matmul(
                            ps, qkvT[:, 2 + h, jb * P:(jb + 1) * P],
                            qkvT[:, h, a * 512:(a + 1) * 512],
                            start=True, stop=(t < 0))
                        if t >= 0:
                            nc.tensor.matmul(ps, ident, masksT[t],
                                             start=False, stop=True)
                        ptt = ptp.tile([P, 512], BF)
                        nc.scalar.activation(ptt, ps, AF.Exp)
                        nc.tensor.matmul(lp, ones_bf, ptt,
                                         start=(jb == 0), stop=(jb == nj - 1))
                        nc.tensor.matmul(
                            po, v_sb[:, jb, h * DH:(h + 1) * DH], ptt,
                            start=(jb == 0), stop=(jb == nj - 1))
                    rl = stat.tile([1, 512], F32, tag="rl")
                    nc.vector.reciprocal(rl, lp)
                    rlps = ps_mm.tile([P, 512], F32, tag="pm")
                    nc.tensor.matmul(rlps, ones_row, rl, start=True, stop=True)
                    rlb = cast.tile([P, 512], F32, tag="rlb")
                    nc.vector.tensor_copy(rlb, rlps)
                    o512 = cast.tile([P, 512], A2DT, tag="o512")
                    nc.vector.tensor_mul(o512, po, rlb)
                    for dd in range(2):
                        d = 2 * a + dd
                        nc.sync.dma_start(
                            a2a_ins[h][d * DH:(d + 1) * DH, :],
                            o512[:, dd * 256:(dd + 1) * 256])
                nc.gpsimd.collective_compute(
                    "AllToAll", mybir.AluOpType.bypass,
                    replica_groups=[[0, 1, 2, 3, 4, 5, 6, 7]],
                    ins=[a2a_ins[h][:, :].opt()],
                    outs=[a2a_outs[h][:, :].opt()])

            # ---- output projection: 256 rows for each batch ----
            # ofT k-index (h, c): global f block = c*2 + h  (f = (c, h, dh))
            for b in range(2):
                ofT = big.tile([P, HC, 4, 256], BF, tag="ofT")
                for h in range(HC):
                    if A2DT == BF:
                        nc.sync.dma_start(
                            ofT[:, h], a2a_outs[h][4 * b * DH:(4 * b + 4) * DH, :]
                            .rearrange("(c p) s -> p c s", p=P))
                    else:
                        of32 = cast.tile([P, 4, 256], F32, tag="of32")
                        nc.sync.dma_start(
                            of32, a2a_outs[h][4 * b * DH:(4 * b + 4) * DH, :]
                            .rearrange("(c p) s -> p c s", p=P))
                        nc.vector.tensor_copy(ofT[:, h], of32)
                for sb in range(2):
                    for cc in range(2):
                        pm = ps_mm.tile([P, 512], F32, tag="pm")
                        kk = 0
                        for c in range(4):
                            for h in range(HC):
                                nc.tensor.matmul(
                                    pm, ofT[:, h, c, sb * P:(sb + 1) * P],
                                    woT[:, c * 2 + h, cc * 512:(cc + 1) * 512],
                                    start=(kk == 0), stop=(kk == KD - 1))
                                kk += 1
                        y = cast.tile([P, 512], F32, tag="y")
                        nc.vector.tensor_copy(y, pm)
                        nc.sync.dma_start(
                            out_ext[b * 256 + sb * P: b * 256 + (sb + 1) * P,
                                    cc * 512:(cc + 1) * 512], y)


def build():
    nc = bacc.Bacc(None, target_bir_lowering=False)
    with tile.TileContext(nc) as tc:
        _body(tc)
    nc.compile()
    return nc


_NC = None


def make_in_maps(inputs):
    x = np.ascontiguousarray(np.asarray(inputs["x"], np.float32))
    gamma = np.asarray(inputs["gamma"], np.float32)
    w_qkv = np.asarray(inputs["w_qkv"], np.float32)
    w_out = np.asarray(inputs["w_out"], np.float32)
    w_prep = w_qkv * gamma[None, :]          # fold RMSNorm gamma
    in_maps = []
    for i in range(8):
        b, g = i // 4, i % 4
        rows = np.concatenate([
            w_prep[256 * g:256 * (g + 1)] * SA,   # fold attn scale into Q
            w_prep[1024 + 256 * g:1024 + 256 * (g + 1)],
            w_prep[2048 + 256 * g:2048 + 256 * (g + 1)]], axis=0)
        import ml_dtypes
        in_maps.append({
            "x": np.ascontiguousarray(x[b]).astype(ml_dtypes.bfloat16),
            "w_qkvT": np.ascontiguousarray(rows.T).astype(ml_dtypes.bfloat16),
            "w_outT": np.ascontiguousarray(w_out.T).astype(ml_dtypes.bfloat16)})
    return in_maps


def run(inputs, trace=False):
    global _NC
    if _NC is None:
        _NC = build()
    in_maps = make_in_maps(inputs)
    br = run_bass_kernel_spmd(_NC, in_maps, list(range(8)), trace=trace)
    out = np.empty((2, S, D), np.float32)
    for i in range(8):
        o = br.results[i]["out"]
        out[0, i * 256:(i + 1) * 256, :] = o[:256]
        out[1, i * 256:(i + 1) * 256, :] = o[256:]
    return out, br


def kernel(**inputs):
    out, _ = run(inputs, trace=False)
    return out
